# revision 4
# baseline (speedup 1.0000x reference)
"""Multi-head attention Trainium2 kernel (8 NeuronCores).

Sharding: core c = (batch b = c // 4, head-group hg = c % 4).
Each core computes 4 heads (256 of the 1024 d_model columns) for one batch:
  - Q^T/K^T projections in transposed layout [dh, seq] (bias via ACT per-partition)
  - V projection in natural layout [seq, dh] (bias folded into bo on host)
  - scores = Q @ K^T + additive mask (mask added pre-exp via PE identity-matmul
    accumulation into PSUM), softmax without max-subtraction (scores are small;
    masked lanes hit exp underflow -> exact 0, matching the reference)
  - denominator via ACT accum_out during the exp pass (free row-sum)
  - attn (normalized, bf16) -> cast-DMA to f32 HBM; DMA-transpose -> attn^T
  - U^T = V^T @ attn^T per head; out^T = Wo^T @ A^T (row-split out_proj)
Host gathers: out = sum of 4 partial out^T per batch (transposed) + bo + bv@wo.
"""

import numpy as np
import ml_dtypes

B = 2
SQ = 2048
SKV = 2048
D = 1024
H = 16
DH = 64
HPC = 4          # heads per core
CPC = HPC * DH   # d_model columns per core = 256
NCORES = 8

BF16 = ml_dtypes.bfloat16

_cache = {}


def _build_nc():
    import concourse.bass as bass
    import concourse.mybir as mybir
    from concourse import tile
    from concourse import bacc

    f32 = mybir.dt.float32
    bf16 = mybir.dt.bfloat16

    nc = bacc.Bacc("TRN2", target_bir_lowering=False, debug=False)

    # ---- DRAM I/O ----
    qT_d = nc.dram_tensor("qT", (D, SQ), bf16, kind="ExternalInput")
    kT_d = nc.dram_tensor("kT", (D, SKV), bf16, kind="ExternalInput")
    vT_d = nc.dram_tensor("vT", (D, SKV), bf16, kind="ExternalInput")
    wq_d = nc.dram_tensor("wq", (D, CPC), bf16, kind="ExternalInput")
    wk_d = nc.dram_tensor("wk", (D, CPC), bf16, kind="ExternalInput")
    wv_d = nc.dram_tensor("wv", (D, CPC), bf16, kind="ExternalInput")
    wo_d = nc.dram_tensor("wo", (CPC, D), bf16, kind="ExternalInput")
    bq_d = nc.dram_tensor("bq", (128, 2), f32, kind="ExternalInput")
    bk_d = nc.dram_tensor("bk", (128, 2), f32, kind="ExternalInput")
    mask_d = nc.dram_tensor("maskadd", (SQ, SKV), bf16, kind="ExternalInput")

    attn_d = nc.dram_tensor("attn_o", (HPC, SQ, SKV), f32, kind="ExternalOutput")
    outT_d = nc.dram_tensor("outT_o", (D, SQ), f32, kind="ExternalOutput")

    NQ = SQ // 128            # 16 q blocks
    NJ = SKV // 128           # 16 kv blocks
    KT_D = D // 128           # 8 k tiles of d_model

    with tile.TileContext(nc) as tc:
        with (
            tc.tile_pool(name="const", bufs=1) as const,
            tc.tile_pool(name="persist", bufs=1) as persist,
            tc.tile_pool(name="xp", bufs=10) as xp,
            tc.tile_pool(name="mask", bufs=2) as maskp,
            tc.tile_pool(name="e", bufs=3) as ep,
            tc.tile_pool(name="ab", bufs=3) as abp,
            tc.tile_pool(name="aT", bufs=2) as aTp,
            tc.tile_pool(name="den", bufs=6) as denp,
            tc.tile_pool(name="at_stage", bufs=2) as atp,
            tc.tile_pool(name="ot", bufs=3) as otp,
            tc.tile_pool(name="ps", bufs=5, space="PSUM") as psp,
            tc.tile_pool(name="psu", bufs=2, space="PSUM") as psup,
        ):
            # ---- constants ----
            ident = const.tile([128, 128], bf16)
            nc.gpsimd.memset(ident, 0.0)
            nc.gpsimd.affine_select(
                out=ident, in_=ident,
                compare_op=mybir.AluOpType.not_equal,
                fill=1.0, base=0, pattern=[[-1, 128]], channel_multiplier=1,
            )

            wq_sb = const.tile([128, KT_D, CPC], bf16, tag="wq")
            wk_sb = const.tile([128, KT_D, CPC], bf16, tag="wk")
            wv_sb = const.tile([128, KT_D, CPC], bf16, tag="wv")
            wo_sb = const.tile([128, 2, D], bf16, tag="wo")
            bq_sb = const.tile([128, 2], f32, tag="bq")
            bk_sb = const.tile([128, 2], f32, tag="bk")
            nc.sync.dma_start(wq_sb[:], wq_d.rearrange("(k p) m -> p k m", p=128))
            nc.sync.dma_start(wk_sb[:], wk_d.rearrange("(k p) m -> p k m", p=128))
            nc.sync.dma_start(wv_sb[:], wv_d.rearrange("(k p) m -> p k m", p=128))
            nc.sync.dma_start(wo_sb[:], wo_d.rearrange("(k p) m -> p k m", p=128))
            nc.sync.dma_start(bq_sb[:], bq_d[:])
            nc.sync.dma_start(bk_sb[:], bk_d[:])

            # persistent projected tensors
            QT = [persist.tile([128, SQ], bf16, tag=f"QT{t}", name=f"QT{t}") for t in range(2)]
            KT = [persist.tile([128, SKV], bf16, tag=f"KT{t}", name=f"KT{t}") for t in range(2)]
            V_sb = persist.tile([128, NJ, CPC], bf16, tag="V")

            # ---- projections: Q^T, K^T ----
            for (x_d, w_sb, b_sb, dst) in (
                (qT_d, wq_sb, bq_sb, QT),
                (kT_d, wk_sb, bk_sb, KT),
            ):
                xk = []
                for k in range(KT_D):
                    t_ = xp.tile([128, SQ], bf16)
                    nc.sync.dma_start(t_, x_d[k * 128:(k + 1) * 128, :])
                    xk.append(t_)
                for t in range(2):
                    pss = [psp.tile([128, 512], f32, name=f"pss{n_}", tag="ps") for n_ in range(4)]
                    for k in range(KT_D):
                        lhsT = w_sb[:, k, t * 128:(t + 1) * 128]
                        for n in range(4):
                            nc.tensor.matmul(
                                pss[n], lhsT, xk[k][:, n * 512:(n + 1) * 512],
                                start=(k == 0), stop=(k == KT_D - 1),
                            )
                    for n in range(4):
                        nc.scalar.activation(
                            dst[t][:, n * 512:(n + 1) * 512], pss[n],
                            mybir.ActivationFunctionType.Identity,
                            bias=b_sb[:, t:t + 1], scale=1.0,
                        )

            # ---- projection: V (natural layout) ----
            xk = []
            for k in range(KT_D):
                t_ = xp.tile([128, SKV], bf16)
                nc.sync.dma_start(t_, vT_d[k * 128:(k + 1) * 128, :])
                xk.append(t_)
            for j in range(NJ):
                psv = psp.tile([128, 512], f32, tag="ps")
                for k in range(KT_D):
                    nc.tensor.matmul(
                        psv[:, :CPC], xk[k][:, j * 128:(j + 1) * 128],
                        wv_sb[:, k, :],
                        start=(k == 0), stop=(k == KT_D - 1),
                    )
                nc.scalar.copy(V_sb[:, j, :], psv[:, :CPC])

            # ---- attention ----
            for qi in range(NQ):
                mt = maskp.tile([128, SKV], bf16)
                nc.sync.dma_start(mt, mask_d[qi * 128:(qi + 1) * 128, :])
                qg, qo = qi // 4, qi % 4
                if qo == 0:
                    AT = atp.tile([128, 2, 512], bf16)
                for h in range(HPC):
                    t, po = h // 2, (h % 2) * 64
                    e = ep.tile([128, SKV], bf16)
                    den4 = denp.tile([128, 4], f32, tag="den4")
                    for n in range(4):
                        ps = psp.tile([128, 512], f32, tag="ps")
                        nc.tensor.matmul(
                            ps, ident, mt[:, n * 512:(n + 1) * 512],
                            start=True, stop=False,
                        )
                        nc.tensor.matmul(
                            ps,
                            QT[t][po:po + 64, qi * 128:(qi + 1) * 128],
                            KT[t][po:po + 64, n * 512:(n + 1) * 512],
                            start=False, stop=True,
                        )
                        nc.scalar.activation(
                            e[:, n * 512:(n + 1) * 512], ps,
                            mybir.ActivationFunctionType.Exp,
                            bias=0.0, scale=0.125,
                            accum_out=den4[:, n:n + 1],
                        )
                    den = denp.tile([128, 1], f32, tag="den")
                    nc.vector.tensor_reduce(
                        den, den4, axis=mybir.AxisListType.X,
                        op=mybir.AluOpType.add,
                    )
                    recip = denp.tile([128, 1], f32, tag="recip")
                    nc.vector.reciprocal(recip, den)
                    ab = abp.tile([128, SKV], bf16)
                    nc.vector.tensor_scalar_mul(ab, e, recip)
                    # f32 attention weights to HBM (SWDGE casts bf16 -> f32)
                    nc.gpsimd.dma_start(
                        attn_d[h, qi * 128:(qi + 1) * 128, :], ab
                    )
                    # transpose attn block-wise for the attn @ V matmul
                    aT = aTp.tile([128, NJ, 128], bf16)
                    for j in range(NJ):
                        nc.sync.dma_start(
                            aT[:, j, :], ab[:, j * 128:(j + 1) * 128],
                            transpose=True,
                        )
                    pu = psup.tile([64, 128], f32)
                    for j in range(NJ):
                        nc.tensor.matmul(
                            pu, V_sb[:, j, h * 64:(h + 1) * 64], aT[:, j, :],
                            start=(j == 0), stop=(j == NJ - 1),
                        )
                    nc.scalar.copy(
                        AT[po:po + 64, t, qo * 128:(qo + 1) * 128], pu
                    )
                # ---- out_proj for each completed group of 4 q-blocks ----
                if qo == 3:
                    for m in range(8):
                        pso = psp.tile([128, 512], f32, tag="ps")
                        for kk in range(2):
                            nc.tensor.matmul(
                                pso, wo_sb[:, kk, m * 128:(m + 1) * 128],
                                AT[:, kk, :],
                                start=(kk == 0), stop=(kk == 1),
                            )
                        ot = otp.tile([128, 512], f32)
                        nc.scalar.copy(ot, pso)
                        nc.sync.dma_start(
                            outT_d[m * 128:(m + 1) * 128,
                                   qg * 512:(qg + 1) * 512], ot
                        )

    nc.compile()
    return nc


def _get_nc():
    if "nc" not in _cache:
        _cache["nc"] = _build_nc()
    return _cache["nc"]


def _prep_in_maps(query, key, value, mask, wq, bq, wk, bk, wv, bv, wo, bo):
    in_maps = []
    maskadd = [
        np.where(mask[b] == 0, np.float32(-1e9), np.float32(0.0)).astype(BF16)
        for b in range(B)
    ]
    qT = [np.ascontiguousarray(query[b].T).astype(BF16) for b in range(B)]
    kT = [np.ascontiguousarray(key[b].T).astype(BF16) for b in range(B)]
    vT = [np.ascontiguousarray(value[b].T).astype(BF16) for b in range(B)]
    for c in range(NCORES):
        b, hg = c // 4, c % 4
        cs = slice(hg * CPC, (hg + 1) * CPC)
        in_maps.append({
            "qT": qT[b],
            "kT": kT[b],
            "vT": vT[b],
            "wq": np.ascontiguousarray(wq[:, cs]).astype(BF16),
            "wk": np.ascontiguousarray(wk[:, cs]).astype(BF16),
            "wv": np.ascontiguousarray(wv[:, cs]).astype(BF16),
            "wo": np.ascontiguousarray(wo[cs, :]).astype(BF16),
            "bq": np.ascontiguousarray(bq[cs].reshape(2, 128).T).astype(np.float32),
            "bk": np.ascontiguousarray(bk[cs].reshape(2, 128).T).astype(np.float32),
            "maskadd": maskadd[b],
        })
    return in_maps


def kernel(query, key, value, mask, wq, bq, wk, bk, wv, bv, wo, bo, _trace=False):
    from concourse.bass_utils import run_bass_kernel_spmd

    query = np.asarray(query, dtype=np.float32)
    key = np.asarray(key, dtype=np.float32)
    value = np.asarray(value, dtype=np.float32)
    mask = np.asarray(mask)
    wq = np.asarray(wq, dtype=np.float32)
    bq = np.asarray(bq, dtype=np.float32)
    wk = np.asarray(wk, dtype=np.float32)
    bk = np.asarray(bk, dtype=np.float32)
    wv = np.asarray(wv, dtype=np.float32)
    bv = np.asarray(bv, dtype=np.float32)
    wo = np.asarray(wo, dtype=np.float32)
    bo = np.asarray(bo, dtype=np.float32)

    nc = _get_nc()
    in_maps = _prep_in_maps(query, key, value, mask,
                            wq, bq, wk, bk, wv, bv, wo, bo)
    res = run_bass_kernel_spmd(
        nc, in_maps, core_ids=list(range(NCORES)), trace=_trace,
    )
    _cache["last_result"] = res

    attn = np.empty((B, H, SQ, SKV), dtype=np.float32)
    out = np.zeros((B, SQ, D), dtype=np.float32)
    for c in range(NCORES):
        b, hg = c // 4, c % 4
        r = res.results[c]
        attn[b, hg * HPC:(hg + 1) * HPC] = r["attn_o"]
        out[b] += r["outT_o"].T
    # bv folded: attn rows sum to 1 -> V bias contributes bv @ wo to every row
    out += bo + bv.astype(np.float32) @ wo
    return out, attn
